# revision 1
# baseline (speedup 1.0000x reference)
"""Trainium2 Bass kernel for nn_Decoder (dense transformer decoder block).

Strategy (8 NeuronCores, two SPMD launches, no collectives):
  L1: tensor-parallel over heads (2 heads/core). Each core embeds all
      4096 tokens (indirect-DMA gather + pos add), transposes h to
      [C, tokens] layout, projects Q/K/V for its 2 heads, and runs causal
      attention with the softmax denominator folded into the AV matmul as
      an appended ones-column of V. Output: yT slice [128, 4096] bf16.
  host: concatenates the 8 yT slices -> yT [1024, 4096] (pure reshaping).
  L2: vocab-parallel logits GEMM. Each core computes
      logits[:, c*4000:(c+1)*4000] = yT.T @ w_head[:, slice] (+ b_head).
  All matmul operands bf16 (fp32 accumulation in PSUM); output fp32.
"""
import numpy as np
import ml_dtypes
import concourse.bass as bass
import concourse.bacc as bacc
import concourse.mybir as mybir
from concourse.tile import TileContext
from concourse.masks import make_identity
from concourse.bass_utils import run_bass_kernel_spmd

BF16 = mybir.dt.bfloat16
F32 = mybir.dt.float32
I32 = mybir.dt.int32
AF = mybir.ActivationFunctionType

B, T, C, H, HS = 2, 2048, 1024, 16, 64
V = 32000
N_CORES = 8
VSL = V // N_CORES  # 4000 vocab columns per core
SCALE = float(C) ** -0.5
NEG = -960.0


def _build_l1(with_bias_qkv, with_bv):
    nc = bacc.Bacc("TRN2", target_bir_lowering=False, debug=False,
                   num_devices=N_CORES)
    tok = nc.dram_tensor("tok_emb_b", [V, C], BF16, kind="ExternalInput")
    pos = nc.dram_tensor("pos_emb_b", [T, C], BF16, kind="ExternalInput")
    idx = nc.dram_tensor("idx", [128, 32], I32, kind="ExternalInput")
    wq = nc.dram_tensor("wq_s", [128, 8, 128], BF16, kind="ExternalInput")
    wk = nc.dram_tensor("wk_s", [128, 8, 128], BF16, kind="ExternalInput")
    wv = nc.dram_tensor("wv_s", [128, 8, 128], BF16, kind="ExternalInput")
    bq = nc.dram_tensor("bq_s", [128, 1], F32, kind="ExternalInput")
    bk = nc.dram_tensor("bk_s", [128, 1], F32, kind="ExternalInput")
    bv = nc.dram_tensor("bv_s", [128, 1], F32, kind="ExternalInput")
    masks = nc.dram_tensor("masks", [128, 4, 512], F32, kind="ExternalInput")
    y_out = nc.dram_tensor("y_out", [128, B * T], BF16, kind="ExternalOutput")

    with TileContext(nc) as tc:
        with (
            tc.tile_pool(name="const", bufs=1) as const,
            tc.tile_pool(name="big", bufs=1) as big,
            tc.tile_pool(name="dram", bufs=1, space="DRAM") as dram,
        ):
            ident = const.tile([128, 128], BF16, name="ident")
            make_identity(nc, ident[:])
            ones1 = const.tile([1, 64], F32, name="ones1")
            nc.gpsimd.memset(ones1[:], 1.0)
            masks_sb = const.tile([128, 4, 512], F32, name="masks_sb")
            nc.sync.dma_start(masks_sb[:], masks.ap())
            bq_sb = const.tile([128, 1], F32, name="bq_sb")
            nc.sync.dma_start(bq_sb[:], bq.ap())
            bk_sb = const.tile([128, 1], F32, name="bk_sb")
            nc.sync.dma_start(bk_sb[:], bk.ap())
            bv_sb = const.tile([128, 1], F32, name="bv_sb")
            nc.sync.dma_start(bv_sb[:], bv.ap())
            idx_sb = const.tile([128, 32], I32, name="idx_sb")
            nc.sync.dma_start(idx_sb[:], idx.ap())
            pos_all = big.tile([128, 16, C], BF16, name="pos_all")
            nc.sync.dma_start(
                pos_all[:], pos.ap().rearrange("(pb p) c -> p pb c", p=128))
            wq_sb = const.tile([128, 8, 128], BF16, name="wq_sb")
            nc.sync.dma_start(wq_sb[:], wq.ap())
            wk_sb = const.tile([128, 8, 128], BF16, name="wk_sb")
            nc.sync.dma_start(wk_sb[:], wk.ap())
            wv_sb = const.tile([128, 8, 128], BF16, name="wv_sb")
            nc.sync.dma_start(wv_sb[:], wv.ap())

            hT = big.tile([128, 8, B * T], BF16, name="hT")
            qT_sb = big.tile([128, B * T], BF16, name="qT_sb")
            kT_sb = big.tile([128, B * T], BF16, name="kT_sb")
            v_sb = big.tile([128, 32, 130], BF16, name="v_sb")
            nc.vector.memset(v_sb[:, :, 64:65], 1.0)
            nc.vector.memset(v_sb[:, :, 129:130], 1.0)

            # Phase E: gather + pos add -> h chunk in DRAM; XBAR transpose
            # back into hT [C-chunk, tokens].
            h_c = [dram.tile([128, C], BF16, name=f"h_c{i}") for i in range(32)]
            with tc.tile_pool(name="gp", bufs=4) as gp:
                for i in range(32):
                    pb = i % 16
                    g = gp.tile([128, C], BF16, tag="g", name="g")
                    nc.gpsimd.indirect_dma_start(
                        out=g[:], out_offset=None,
                        in_=tok.ap(),
                        in_offset=bass.IndirectOffsetOnAxis(
                            ap=idx_sb[:, i:i + 1], axis=0),
                    )
                    nc.vector.tensor_add(g[:], g[:], pos_all[:, pb, :])
                    nc.sync.dma_start(h_c[i][:], g[:])
                    for cc in range(8):
                        nc.sync.dma_start_transpose(
                            hT[:, cc, i * 128:(i + 1) * 128],
                            h_c[i][:, cc * 128:(cc + 1) * 128])

            # Phase P: QKV projections, both heads at once.
            with (
                tc.tile_pool(name="pp", bufs=4, space="PSUM") as pp,
                tc.tile_pool(name="tpp", bufs=2, space="PSUM") as tpp,
                tc.tile_pool(name="vt", bufs=3) as vtp,
            ):
                for tt in range(8):
                    sl = slice(tt * 512, (tt + 1) * 512)
                    qps = pp.tile([128, 512], F32, tag="proj", name="qps")
                    for cc in range(8):
                        nc.tensor.matmul(qps[:], lhsT=wq_sb[:, cc, :],
                                         rhs=hT[:, cc, sl],
                                         start=(cc == 0), stop=(cc == 7))
                    if with_bias_qkv:
                        nc.scalar.activation(qT_sb[:, sl], qps[:], AF.Identity,
                                             bias=bq_sb[:, 0:1], scale=1.0)
                    else:
                        nc.scalar.copy(qT_sb[:, sl], qps[:])
                    kps = pp.tile([128, 512], F32, tag="proj", name="kps")
                    for cc in range(8):
                        nc.tensor.matmul(kps[:], lhsT=wk_sb[:, cc, :],
                                         rhs=hT[:, cc, sl],
                                         start=(cc == 0), stop=(cc == 7))
                    if with_bias_qkv:
                        nc.scalar.activation(kT_sb[:, sl], kps[:], AF.Identity,
                                             bias=bk_sb[:, 0:1], scale=1.0)
                    else:
                        nc.scalar.copy(kT_sb[:, sl], kps[:])
                    vps = pp.tile([128, 512], F32, tag="proj", name="vps")
                    for cc in range(8):
                        nc.tensor.matmul(vps[:], lhsT=wv_sb[:, cc, :],
                                         rhs=hT[:, cc, sl],
                                         start=(cc == 0), stop=(cc == 7))
                    vtmp = vtp.tile([128, 512], BF16, tag="vtmp", name="vtmp")
                    if with_bv:
                        nc.scalar.activation(vtmp[:], vps[:], AF.Identity,
                                             bias=bv_sb[:, 0:1], scale=1.0)
                    else:
                        nc.vector.tensor_copy(vtmp[:], vps[:])
                    for st in range(4):
                        tps = tpp.tile([128, 128], BF16, tag="tp", name="tps")
                        nc.tensor.transpose(
                            tps[:], vtmp[:, st * 128:(st + 1) * 128], ident[:])
                        ch = tt * 4 + st
                        for h in range(2):
                            nc.vector.tensor_copy(
                                v_sb[:, ch, h * 65:h * 65 + 64],
                                tps[:, h * 64:(h + 1) * 64])

            # Phase A: causal attention per (batch, local head).
            with (
                tc.tile_pool(name="spp", bufs=4, space="PSUM") as spp,
                tc.tile_pool(name="ypp", bufs=2, space="PSUM") as ypp,
                tc.tile_pool(name="rpp", bufs=2, space="PSUM") as rpp,
                tc.tile_pool(name="ap", bufs=4) as apool,
                tc.tile_pool(name="ep", bufs=3) as epool,
            ):
                for u in range(4):
                    b, h = u // 2, u % 2
                    hsl = slice(h * 64, (h + 1) * 64)
                    for qt in range(4):
                        qsl = slice(b * T + qt * 512, b * T + (qt + 1) * 512)
                        yps = ypp.tile([65, 512], F32, tag="yps", name="yps")
                        nkc = 4 * (qt + 1)
                        for kc in range(nkc):
                            ksl = slice(b * T + kc * 128,
                                        b * T + (kc + 1) * 128)
                            sps = spp.tile([128, 512], F32, tag="sps",
                                           name="sps")
                            nc.tensor.matmul(sps[:], lhsT=kT_sb[hsl, ksl],
                                             rhs=qT_sb[hsl, qsl],
                                             start=True, stop=True)
                            if kc >= 4 * qt:
                                nc.vector.tensor_tensor(
                                    sps[:], sps[:],
                                    masks_sb[:, kc - 4 * qt, :],
                                    op=mybir.AluOpType.add)
                            att = apool.tile([128, 512], BF16, tag="att",
                                             name="att")
                            nc.scalar.activation(att[:], sps[:], AF.Exp,
                                                 scale=SCALE)
                            nc.tensor.matmul(
                                yps[:],
                                lhsT=v_sb[:, b * 16 + kc, h * 65:h * 65 + 65],
                                rhs=att[:],
                                start=(kc == 0), stop=(kc == nkc - 1))
                        rec = epool.tile([1, 512], F32, tag="rec", name="rec")
                        nc.vector.reciprocal(rec[:], yps[64:65, :])
                        rbps = rpp.tile([64, 512], F32, tag="rb", name="rbps")
                        nc.tensor.matmul(rbps[:], lhsT=ones1[:], rhs=rec[:],
                                         start=True, stop=True)
                        rb = epool.tile([64, 512], F32, tag="rb_sb", name="rb")
                        nc.scalar.copy(rb[:], rbps[:])
                        yb = epool.tile([64, 512], BF16, tag="yb", name="yb")
                        nc.vector.tensor_mul(yb[:], yps[0:64, :], rb[:])
                        nc.sync.dma_start(y_out.ap()[hsl, qsl], yb[:])
    nc.compile()
    return nc


def _build_l2(with_bias):
    nc = bacc.Bacc("TRN2", target_bir_lowering=False, debug=False,
                   num_devices=N_CORES)
    yT = nc.dram_tensor("yT", [128, 8, B * T], BF16, kind="ExternalInput")
    wh = nc.dram_tensor("wh", [128, 8, VSL], BF16, kind="ExternalInput")
    bh = nc.dram_tensor("bh", [128, VSL], F32, kind="ExternalInput")
    out = nc.dram_tensor("logits", [B * T, VSL], F32, kind="ExternalOutput")
    VT = 500
    NT = (B * T) // 128
    NV = VSL // VT
    GROUP = 4
    with TileContext(nc) as tc:
        with (
            tc.tile_pool(name="big", bufs=1) as big,
            tc.tile_pool(name="outp", bufs=3) as outp,
            tc.tile_pool(name="psum", bufs=8, space="PSUM") as pp,
        ):
            yT_sb = big.tile([128, 8, B * T], BF16, name="yT_sb")
            nc.sync.dma_start(yT_sb[:], yT.ap())
            wh_sb = big.tile([128, 8, VSL], BF16, name="wh_sb")
            nc.sync.dma_start(wh_sb[:], wh.ap())
            if with_bias:
                bh_sb = big.tile([128, VSL], F32, name="bh_sb")
                nc.sync.dma_start(bh_sb[:], bh.ap())
            for tt in range(NT):
                for vg0 in range(0, NV, GROUP):
                    vts = list(range(vg0, min(vg0 + GROUP, NV)))
                    psums = {vt: pp.tile([128, VT], F32, tag="ps",
                                         name=f"ps{vt % GROUP}")
                             for vt in vts}
                    for cc in range(8):
                        for vt in vts:
                            nc.tensor.matmul(
                                psums[vt][:],
                                lhsT=yT_sb[:, cc, tt * 128:(tt + 1) * 128],
                                rhs=wh_sb[:, cc, vt * VT:(vt + 1) * VT],
                                start=(cc == 0), stop=(cc == 7))
                    o = outp.tile([128, len(vts) * VT], F32, tag="o", name="o")
                    for j, vt in enumerate(vts):
                        if with_bias:
                            nc.vector.tensor_add(
                                o[:, j * VT:(j + 1) * VT], psums[vt][:],
                                bh_sb[:, vt * VT:(vt + 1) * VT])
                        else:
                            nc.vector.tensor_copy(
                                o[:, j * VT:(j + 1) * VT], psums[vt][:])
                    nc.sync.dma_start(
                        out.ap()[tt * 128:(tt + 1) * 128,
                                 vg0 * VT:(vg0 + len(vts)) * VT],
                        o[:])
    nc.compile()
    return nc


_CACHE = {}


def _get(key, builder, *a):
    if key not in _CACHE:
        _CACHE[key] = builder(*a)
    return _CACHE[key]


def _l1_inputs(x, tok_emb, pos_emb, wq, bq, wk, bk, wv, bv, core):
    bf = ml_dtypes.bfloat16
    hsel = [2 * core, 2 * core + 1]
    x_i = np.asarray(x).astype(np.int32).reshape(B * T)
    idx = np.ascontiguousarray(x_i.reshape(32, 128).T)

    def wslice(w):
        s = np.asarray(w)[hsel].astype(bf)
        s = np.transpose(s, (1, 0, 2)).reshape(C, 128)
        return np.ascontiguousarray(s.reshape(8, 128, 128).transpose(1, 0, 2))

    def bslice(bias):
        return np.ascontiguousarray(
            np.asarray(bias)[hsel].astype(np.float32).reshape(128, 1))

    i_ = np.arange(128)[:, None]
    j_ = np.arange(512)[None, :]
    m = np.zeros((128, 4, 512), np.float32)
    for v_ in range(4):
        m[:, v_, :] = np.where(128 * v_ + i_ > j_, NEG, 0.0)

    return dict(
        tok_emb_b=np.asarray(tok_emb).astype(bf),
        pos_emb_b=np.asarray(pos_emb).astype(bf),
        idx=idx,
        wq_s=wslice(wq), wk_s=wslice(wk), wv_s=wslice(wv),
        bq_s=bslice(bq), bk_s=bslice(bk), bv_s=bslice(bv),
        masks=m,
    )


def kernel(x, tok_emb, pos_emb, wq, bq, wk, bk, wv, bv, w_head, b_head):
    bf = ml_dtypes.bfloat16
    bias_qkv = bool(np.any(np.asarray(bq)) or np.any(np.asarray(bk)))
    bias_v = bool(np.any(np.asarray(bv)))
    bias_h = bool(np.any(np.asarray(b_head)))

    # ---- L1: heads-parallel attention
    nc1 = _get(("l1", bias_qkv, bias_v), _build_l1, bias_qkv, bias_v)
    ins1 = [_l1_inputs(x, tok_emb, pos_emb, wq, bq, wk, bk, wv, bv, c)
            for c in range(N_CORES)]
    res1 = run_bass_kernel_spmd(nc1, ins1, core_ids=list(range(N_CORES)))
    yT = np.concatenate(
        [np.asarray(res1.results[c]["y_out"]) for c in range(N_CORES)],
        axis=0)  # [1024, 4096] bf16
    yT_in = np.ascontiguousarray(
        yT.reshape(8, 128, B * T).transpose(1, 0, 2))

    # ---- L2: vocab-parallel logits
    nc2 = _get(("l2", bias_h), _build_l2, bias_h)
    wh_b = np.asarray(w_head).astype(bf)
    bh_f = np.asarray(b_head).astype(np.float32)
    ins2 = []
    for c in range(N_CORES):
        whs = np.ascontiguousarray(
            wh_b[:, c * VSL:(c + 1) * VSL]
            .reshape(8, 128, VSL).transpose(1, 0, 2))
        bhs = np.ascontiguousarray(
            np.broadcast_to(bh_f[c * VSL:(c + 1) * VSL], (128, VSL)))
        ins2.append(dict(yT=yT_in, wh=whs, bh=bhs))
    res2 = run_bass_kernel_spmd(nc2, ins2, core_ids=list(range(N_CORES)))
    logits = np.concatenate(
        [res2.results[c]["logits"] for c in range(N_CORES)], axis=1)
    return logits.reshape(B, T, V).astype(np.float32)


# revision 3
# speedup vs baseline: 1.3875x; 1.3875x over previous
"""Trainium2 Bass kernel for nn_Decoder (dense transformer decoder block).

Strategy (8 NeuronCores, two SPMD launches, no collectives):
  L1: tensor-parallel over heads (2 heads/core). Each core embeds all
      4096 tokens (indirect-DMA gather + pos add), transposes h to
      [C, tokens] layout, projects Q/K/V for its 2 heads, and runs causal
      attention with the softmax denominator folded into the AV matmul as
      an appended ones-column of V. Output: yT slice [128, 4096] bf16.
  host: concatenates the 8 yT slices -> yT [1024, 4096] (pure reshaping).
  L2: vocab-parallel logits GEMM. Each core computes
      logits[:, c*4000:(c+1)*4000] = yT.T @ w_head[:, slice] (+ b_head).
  All matmul operands bf16 (fp32 accumulation in PSUM); output fp32.
"""
import numpy as np
import ml_dtypes
import concourse.bass as bass
import concourse.bacc as bacc
import concourse.mybir as mybir
from concourse.tile import TileContext
from concourse.masks import make_identity
from concourse.bass_utils import run_bass_kernel_spmd

BF16 = mybir.dt.bfloat16
F32 = mybir.dt.float32
I32 = mybir.dt.int32
AF = mybir.ActivationFunctionType

B, T, C, H, HS = 2, 2048, 1024, 16, 64
V = 32000
N_CORES = 8
VSL = V // N_CORES  # 4000 vocab columns per core
SCALE = float(C) ** -0.5
NEG = -960.0


def _build_l1(with_bias_qkv, with_bv):
    GB = 16
    nc = bacc.Bacc("TRN2", target_bir_lowering=False, debug=False,
                   num_devices=N_CORES)
    tok = nc.dram_tensor("tok_emb_b", [V, C], BF16, kind="ExternalInput")
    pos = nc.dram_tensor("pos_emb_b", [T, C], BF16, kind="ExternalInput")
    idx = nc.dram_tensor("idx", [128, 32], I32, kind="ExternalInput")
    wq = nc.dram_tensor("wq_s", [128, 8, 128], BF16, kind="ExternalInput")
    wk = nc.dram_tensor("wk_s", [128, 8, 128], BF16, kind="ExternalInput")
    wv = nc.dram_tensor("wv_s", [128, 8, 128], BF16, kind="ExternalInput")
    bq = nc.dram_tensor("bq_s", [128, 1], F32, kind="ExternalInput")
    bk = nc.dram_tensor("bk_s", [128, 1], F32, kind="ExternalInput")
    bv = nc.dram_tensor("bv_s", [128, 1], F32, kind="ExternalInput")
    masks = nc.dram_tensor("masks_b", [128, 4, 512], BF16,
                           kind="ExternalInput")
    y_out = nc.dram_tensor("y_out", [128, B * T], BF16, kind="ExternalOutput")

    with TileContext(nc) as tc:
        with (
            tc.tile_pool(name="const", bufs=1) as const,
            tc.tile_pool(name="big", bufs=1) as big,
            tc.tile_pool(name="dram", bufs=1, space="DRAM") as dram,
            tc.tile_pool(name="gp", bufs=GB) as gp,
            tc.tile_pool(name="pp", bufs=2, space="PSUM") as pp,
            tc.tile_pool(name="tpp", bufs=1, space="PSUM") as tpp,
            tc.tile_pool(name="vt", bufs=3) as vtp,
            tc.tile_pool(name="spp", bufs=3, space="PSUM") as spp,
            tc.tile_pool(name="ypp", bufs=2, space="PSUM") as ypp,
            tc.tile_pool(name="ap", bufs=4) as apool,
            tc.tile_pool(name="ep", bufs=3) as epool,
        ):
            ident = const.tile([128, 128], BF16, name="ident")
            make_identity(nc, ident[:])
            ones1 = const.tile([1, 64], F32, name="ones1")
            nc.gpsimd.memset(ones1[:], 1.0)
            masks_sb = const.tile([128, 4, 512], BF16, name="masks_sb")
            nc.sync.dma_start(masks_sb[:], masks.ap())
            bq_sb = const.tile([128, 1], F32, name="bq_sb")
            nc.sync.dma_start(bq_sb[:], bq.ap())
            bk_sb = const.tile([128, 1], F32, name="bk_sb")
            nc.sync.dma_start(bk_sb[:], bk.ap())
            bv_sb = const.tile([128, 1], F32, name="bv_sb")
            nc.sync.dma_start(bv_sb[:], bv.ap())
            idx_sb = const.tile([128, 32], I32, name="idx_sb")
            nc.sync.dma_start(idx_sb[:], idx.ap())
            pos_all = big.tile([128, 16, C], BF16, name="pos_all")
            nc.sync.dma_start(
                pos_all[:], pos.ap().rearrange("(pb p) c -> p pb c", p=128))
            wq_sb = const.tile([128, 8, 128], BF16, name="wq_sb")
            nc.sync.dma_start(wq_sb[:], wq.ap())
            wk_sb = const.tile([128, 8, 128], BF16, name="wk_sb")
            nc.sync.dma_start(wk_sb[:], wk.ap())
            wv_sb = const.tile([128, 8, 128], BF16, name="wv_sb")
            nc.sync.dma_start(wv_sb[:], wv.ap())

            hT_t = [big.tile([128, 8, 512], BF16, name=f"hT{t}")
                    for t in range(8)]
            qT_t = [big.tile([128, 512], BF16, name=f"qT{t}")
                    for t in range(8)]
            kT_t = [big.tile([128, 512], BF16, name=f"kT{t}")
                    for t in range(8)]
            v_t = [big.tile([128, 4, 130], BF16, name=f"v{t}")
                   for t in range(8)]
            for t in range(8):
                nc.vector.memset(v_t[t][:, :, 64:65], 1.0)
                nc.vector.memset(v_t[t][:, :, 129:130], 1.0)
            h_g = [dram.tile([512, C], BF16, name=f"h_g{t}")
                   for t in range(8)]

            # all gathers + h writes first: keeps the gather stream dense
            for tt in range(8):
                for j in range(4):
                    i = tt * 4 + j
                    pb = i % 16
                    g = gp.tile([128, C], BF16, tag="g", name="g")
                    nc.gpsimd.indirect_dma_start(
                        out=g[:], out_offset=None,
                        in_=tok.ap(),
                        in_offset=bass.IndirectOffsetOnAxis(
                            ap=idx_sb[:, i:i + 1], axis=0),
                    )
                    nc.vector.tensor_add(g[:], g[:], pos_all[:, pb, :])
                    nc.sync.dma_start(
                        h_g[tt][j * 128:(j + 1) * 128, :], g[:])

            for tt in range(8):
                # xbar transpose h -> hT for this tile
                for cc in range(8):
                    nc.sync.dma_start_transpose(
                        hT_t[tt][:, cc, :],
                        h_g[tt][:, cc * 128:(cc + 1) * 128])
                hT = hT_t[tt]
                # QKV projections (both heads at once)
                qps = pp.tile([128, 512], F32, tag="proj", name="qps")
                for cc in range(8):
                    nc.tensor.matmul(qps[:], lhsT=wq_sb[:, cc, :],
                                     rhs=hT[:, cc, :],
                                     start=(cc == 0), stop=(cc == 7))
                if with_bias_qkv:
                    nc.scalar.activation(qT_t[tt][:], qps[:], AF.Identity,
                                         bias=bq_sb[:, 0:1], scale=1.0)
                else:
                    nc.vector.tensor_copy(qT_t[tt][:], qps[:])
                kps = pp.tile([128, 512], F32, tag="proj", name="kps")
                for cc in range(8):
                    nc.tensor.matmul(kps[:], lhsT=wk_sb[:, cc, :],
                                     rhs=hT[:, cc, :],
                                     start=(cc == 0), stop=(cc == 7))
                if with_bias_qkv:
                    nc.scalar.activation(kT_t[tt][:], kps[:], AF.Identity,
                                         bias=bk_sb[:, 0:1], scale=1.0)
                else:
                    nc.vector.tensor_copy(kT_t[tt][:], kps[:])
                vps = pp.tile([128, 512], F32, tag="proj", name="vps")
                for cc in range(8):
                    nc.tensor.matmul(vps[:], lhsT=wv_sb[:, cc, :],
                                     rhs=hT[:, cc, :],
                                     start=(cc == 0), stop=(cc == 7))
                vtmp = vtp.tile([128, 512], BF16, tag="vtmp", name="vtmp")
                if with_bv:
                    nc.scalar.activation(vtmp[:], vps[:], AF.Identity,
                                         bias=bv_sb[:, 0:1], scale=1.0)
                else:
                    nc.vector.tensor_copy(vtmp[:], vps[:])
                for st in range(4):
                    tps = tpp.tile([128, 128], BF16, tag="tp", name="tps")
                    nc.tensor.transpose(
                        tps[:], vtmp[:, st * 128:(st + 1) * 128], ident[:])
                    for h in range(2):
                        nc.vector.tensor_copy(
                            v_t[tt][:, st, h * 65:h * 65 + 64],
                            tps[:, h * 64:(h + 1) * 64])

                # attention for this query tile (2 local heads)
                b, qt = tt // 4, tt % 4
                for h in range(2):
                    hsl = slice(h * 64, (h + 1) * 64)
                    yps = ypp.tile([65, 512], F32, tag="yps", name="yps")
                    nkc = 4 * (qt + 1)
                    for kc in range(nkc):
                        ktt = b * 4 + kc // 4
                        sps = spp.tile([128, 512], F32, tag="sps", name="sps")
                        nc.tensor.matmul(
                            sps[:],
                            lhsT=kT_t[ktt][hsl,
                                           (kc % 4) * 128:(kc % 4 + 1) * 128],
                            rhs=qT_t[b * 4 + qt][hsl, :],
                            start=True, stop=True)
                        att = apool.tile([128, 512], BF16, tag="att",
                                         name="att")
                        nc.scalar.activation(att[:], sps[:], AF.Exp,
                                             scale=SCALE)
                        if kc >= 4 * qt:
                            nc.vector.tensor_mul(
                                att[:], att[:], masks_sb[:, kc - 4 * qt, :])
                        nc.tensor.matmul(
                            yps[:],
                            lhsT=v_t[ktt][:, kc % 4, h * 65:h * 65 + 65],
                            rhs=att[:],
                            start=(kc == 0), stop=(kc == nkc - 1))
                    rec = epool.tile([1, 512], F32, tag="rec", name="rec")
                    nc.vector.reciprocal(rec[:], yps[64:65, :])
                    rbps = spp.tile([64, 512], F32, tag="sps", name="rbps")
                    nc.tensor.matmul(rbps[:], lhsT=ones1[:], rhs=rec[:],
                                     start=True, stop=True)
                    rb = epool.tile([64, 512], F32, tag="rb_sb", name="rb")
                    nc.vector.tensor_copy(rb[:], rbps[:])
                    yb = epool.tile([64, 512], BF16, tag="yb", name="yb")
                    nc.vector.tensor_mul(yb[:], yps[0:64, :], rb[:])
                    qsl = slice(b * T + qt * 512, b * T + (qt + 1) * 512)
                    nc.sync.dma_start(y_out.ap()[hsl, qsl], yb[:])
    nc.compile()
    return nc


def _build_l2(with_bias):
    nc = bacc.Bacc("TRN2", target_bir_lowering=False, debug=False,
                   num_devices=N_CORES)
    yT = nc.dram_tensor("yT", [128, 8, B * T], BF16, kind="ExternalInput")
    wh = nc.dram_tensor("wh", [128, 8, VSL], BF16, kind="ExternalInput")
    bh = nc.dram_tensor("bh", [128, VSL], F32, kind="ExternalInput")
    out = nc.dram_tensor("logits", [B * T, VSL], F32, kind="ExternalOutput")
    VT = 500
    NT = (B * T) // 128
    NV = VSL // VT
    GROUP = 4
    with TileContext(nc) as tc:
        with (
            tc.tile_pool(name="big", bufs=1) as big,
            tc.tile_pool(name="outp", bufs=3) as outp,
            tc.tile_pool(name="psum", bufs=8, space="PSUM") as pp,
        ):
            yT_sb = big.tile([128, 8, B * T], BF16, name="yT_sb")
            nc.sync.dma_start(yT_sb[:], yT.ap())
            wh_sb = big.tile([128, 8, VSL], BF16, name="wh_sb")
            for vt in range(NV):
                nc.sync.dma_start(wh_sb[:, :, vt * VT:(vt + 1) * VT],
                                  wh.ap()[:, :, vt * VT:(vt + 1) * VT])
            if with_bias:
                bh_sb = big.tile([128, VSL], F32, name="bh_sb")
                nc.sync.dma_start(bh_sb[:], bh.ap())
            for tt in range(NT):
                for vg0 in range(0, NV, GROUP):
                    vts = list(range(vg0, min(vg0 + GROUP, NV)))
                    psums = {vt: pp.tile([128, VT], F32, tag="ps",
                                         name=f"ps{vt % GROUP}")
                             for vt in vts}
                    for cc in range(8):
                        for vt in vts:
                            nc.tensor.matmul(
                                psums[vt][:],
                                lhsT=yT_sb[:, cc, tt * 128:(tt + 1) * 128],
                                rhs=wh_sb[:, cc, vt * VT:(vt + 1) * VT],
                                start=(cc == 0), stop=(cc == 7))
                    o = outp.tile([128, len(vts) * VT], F32, tag="o", name="o")
                    for j, vt in enumerate(vts):
                        if with_bias:
                            nc.vector.tensor_add(
                                o[:, j * VT:(j + 1) * VT], psums[vt][:],
                                bh_sb[:, vt * VT:(vt + 1) * VT])
                        else:
                            nc.vector.tensor_copy(
                                o[:, j * VT:(j + 1) * VT], psums[vt][:])
                    nc.sync.dma_start(
                        out.ap()[tt * 128:(tt + 1) * 128,
                                 vg0 * VT:(vg0 + len(vts)) * VT],
                        o[:])
    nc.compile()
    return nc


_CACHE = {}


def _get(key, builder, *a):
    if key not in _CACHE:
        _CACHE[key] = builder(*a)
    return _CACHE[key]


def _l1_inputs(x, tok_emb, pos_emb, wq, bq, wk, bk, wv, bv, core):
    bf = ml_dtypes.bfloat16
    hsel = [2 * core, 2 * core + 1]
    x_i = np.asarray(x).astype(np.int32).reshape(B * T)
    idx = np.ascontiguousarray(x_i.reshape(32, 128).T)

    def wslice(w):
        s = np.asarray(w)[hsel].astype(bf)
        s = np.transpose(s, (1, 0, 2)).reshape(C, 128)
        return np.ascontiguousarray(s.reshape(8, 128, 128).transpose(1, 0, 2))

    def bslice(bias):
        return np.ascontiguousarray(
            np.asarray(bias)[hsel].astype(np.float32).reshape(128, 1))

    i_ = np.arange(128)[:, None]
    j_ = np.arange(512)[None, :]
    m = np.zeros((128, 4, 512), np.float32)
    for v_ in range(4):
        m[:, v_, :] = np.where(128 * v_ + i_ > j_, 0.0, 1.0)
    m = m.astype(ml_dtypes.bfloat16)

    return dict(
        tok_emb_b=np.asarray(tok_emb).astype(bf),
        pos_emb_b=np.asarray(pos_emb).astype(bf),
        idx=idx,
        wq_s=wslice(wq), wk_s=wslice(wk), wv_s=wslice(wv),
        bq_s=bslice(bq), bk_s=bslice(bk), bv_s=bslice(bv),
        masks_b=m,
    )


def kernel(x, tok_emb, pos_emb, wq, bq, wk, bk, wv, bv, w_head, b_head):
    bf = ml_dtypes.bfloat16
    bias_qkv = bool(np.any(np.asarray(bq)) or np.any(np.asarray(bk)))
    bias_v = bool(np.any(np.asarray(bv)))
    bias_h = bool(np.any(np.asarray(b_head)))

    # ---- L1: heads-parallel attention
    nc1 = _get(("l1", bias_qkv, bias_v), _build_l1, bias_qkv, bias_v)
    ins1 = [_l1_inputs(x, tok_emb, pos_emb, wq, bq, wk, bk, wv, bv, c)
            for c in range(N_CORES)]
    res1 = run_bass_kernel_spmd(nc1, ins1, core_ids=list(range(N_CORES)))
    yT = np.concatenate(
        [np.asarray(res1.results[c]["y_out"]) for c in range(N_CORES)],
        axis=0)  # [1024, 4096] bf16
    yT_in = np.ascontiguousarray(
        yT.reshape(8, 128, B * T).transpose(1, 0, 2))

    # ---- L2: vocab-parallel logits
    nc2 = _get(("l2", bias_h), _build_l2, bias_h)
    wh_b = np.asarray(w_head).astype(bf)
    bh_f = np.asarray(b_head).astype(np.float32)
    ins2 = []
    for c in range(N_CORES):
        whs = np.ascontiguousarray(
            wh_b[:, c * VSL:(c + 1) * VSL]
            .reshape(8, 128, VSL).transpose(1, 0, 2))
        bhs = np.ascontiguousarray(
            np.broadcast_to(bh_f[c * VSL:(c + 1) * VSL], (128, VSL)))
        ins2.append(dict(yT=yT_in, wh=whs, bh=bhs))
    res2 = run_bass_kernel_spmd(nc2, ins2, core_ids=list(range(N_CORES)))
    logits = np.concatenate(
        [res2.results[c]["logits"] for c in range(N_CORES)], axis=1)
    return logits.reshape(B, T, V).astype(np.float32)
